# revision 14
# baseline (speedup 1.0000x reference)
"""Focal-loss kernel for Trainium2 (Bass/Tile), 8-core data-parallel.

Computes, for fp32 inputs predictions/targets of shape (32, 8400, 720):

    total = sum over 5 heads of
        sum_b mean_{p,d}( -(t*(1-pc)^g*ln(pc) + (1-t)*pc^g*ln(1-pc)) )

with pc = clip(p, 1e-7, 1-1e-7), head splits (160,160,160,160,80) and
gammas (2.5, 2.5, 2.0, 2.0, 3.0).

Final design (HW-measured engine rates drive the work split; both
ACT and DVE land ~91-96% busy at the joint equilibrium):
  - Host pre-casts p to fp16 and t to bf16, halving HBM traffic
    (96.8MB/core, ~280us at ~347GB/s/core).
  - ACT (~1 el/ns/partition): l1p = Ln((1+eps) - p) [exact, fused
    subtract], lp = Ln(p + eps) on channels [0, LP_SPLIT), and
    A = w*aA*(1-p)^g = Exp(g*l1p + ln(w*aA)).
  - DVE tensor_scalar (~2.7-3.6 el/ns): bit-trick fast math off the
    fp16 bit pattern of p:
       lp[LP_SPLIT:] = bits(p)*ln2/1024 - 15*ln2   (fastlog; chord
                                                    error folded in aA)
       bitsB = bits(p)*(g/8) + cB  -> bf16 bits of  B = w*p^g
    The log2/exp2 chord errors are deterministic functions of p; the
    per-range constants aA/cs are root-solved offline on the full fp16
    grid so E[sum] matches the fp64 reference (numpy sim rel err ~4e-7;
    measured on HW 2e-4; the gate is 2e-2).
  - DVE tensor_tensor (~1.8 el/ns, 2x mode): f1 = A*lp, f2 = B*l1p,
    d = f1 - f2 (overwrites A), v = t*d (overwrites B).
    (Pool/gpsimd tensor ops measured net-negative: 0.42 el/ns and they
    slam SBUF, degrading concurrent DVE far more than they contribute.)
  - PE: ones-matmul column sums of v and f2 into PSUM (2 streams).
  - Host: total = -(sum(v) + sum(f2)) over cores in fp64.
  - Small tiles (7 rows/partition) + io bufs=3 keep DMA ahead and cut
    the cross-engine SBUF contention tax (measured 649->546us vs rr=8).

Sharding: rows (b*p flattened: 268800 rows of 720 channels) split
contiguously across 8 cores, 33600 rows each.
"""

import math
import os
from contextlib import ExitStack

import numpy as np
import ml_dtypes

from concourse import bacc, mybir, tile
from concourse.bass_utils import run_bass_kernel_spmd

# Problem constants (hardcoded per harness contract).
B, P, D = 32, 8400, 720
N_CORES = 8
ROWS = B * P                 # 268800
RPC = ROWS // N_CORES        # 33600 rows per core
EPS = 1e-7
LN2 = math.log(2.0)

F32 = mybir.dt.float32
F16 = mybir.dt.float16
BF16 = mybir.dt.bfloat16
I16 = mybir.dt.int16
AF = mybir.ActivationFunctionType
ALU = mybir.AluOpType

# fastlog: ln(p) ~= bits_f16(p)*FL_K + FL_C
FL_K = LN2 / 1024.0
FL_C = -15.0 * LN2

# (c0, c1, gamma, head_width): contiguous channel ranges with constant
# (g, w).  CAL[(g, Dh)] = (aA, cs): offline-calibrated so the expected
# f1/f2 sums over the fp16 grid match the fp64 reference (see module doc).
RANGES = [
    (0, 320, 2.5, 160),
    (320, 640, 2.0, 160),
    (640, 720, 3.0, 80),
]
CAL = {
    (2.5, 160): (0.980244, 4.5250),
    (2.0, 160): (0.978827, 2.3365),
    (3.0, 80): (0.981308, 6.7113),
}
# lp is exact ACT-Ln on channels [0, LP_SPLIT) and DVE fastlog on
# [LP_SPLIT, 720) -- balances the two engines.  A's aA constant differs
# by lp method (the fastlog chord bias is folded into aA).
LP_SPLIT = 560
A_RANGES = [
    (0, 320, 2.5, 160, 0.999973),
    (320, 560, 2.0, 160, 0.999999),
    (560, 640, 2.0, 160, 0.978827),
    (640, 720, 3.0, 80, 0.981308),
]

R_MAIN = 7        # rows per partition per main-loop tile
CHUNK = 504       # matmul moving free-dim chunk (<=512); 7*720 = 10 chunks

_ACT_SET = "natural_log_exp_and_others"
_act_tables_patched = False


def _pin_act_table_set():
    """Make Ln/Exp resolve only to the one table set containing both, so
    the table-load pass emits a single load instead of thrashing."""
    global _act_tables_patched
    if _act_tables_patched:
        return
    orig = bacc.get_activation_tables

    def patched(arch):
        tables = orig(arch)
        pinned = {AF.Ln, AF.Exp}
        assert pinned <= tables[_ACT_SET], tables[_ACT_SET]
        return {
            name: (funcs if name == _ACT_SET else funcs - pinned)
            for name, funcs in tables.items()
        }

    bacc.get_activation_tables = patched
    _act_tables_patched = True


def _iter_plan(rows):
    """Split `rows` into (npart, rows_per_partition) tiles."""
    plan = []
    r = rows
    # two small pipeline-fill tiles first: the first TT chain starts after
    # ~2 rows of DMA+ACT instead of R_MAIN rows
    for rr0 in (2, 4):
        if r >= 128 * (rr0 + R_MAIN):
            plan.append((128, rr0))
            r -= 128 * rr0
    while r >= 128 * R_MAIN:
        plan.append((128, R_MAIN))
        r -= 128 * R_MAIN
    if r >= 128:
        plan.append((128, r // 128))
        r -= 128 * (r // 128)
    if r:
        assert r % 64 == 0, r
        plan.append((r, 1))
    return plan


def _cB(g, Dh):
    w = 1.0 / (P * Dh)
    aA, cs = CAL[(g, Dh)]
    return 16256.0 + 128.0 * (math.log2(w) - 15.0 * g) + cs


def build_program(rows_per_core=RPC):
    _pin_act_table_set()
    nc = bacc.Bacc("TRN2", target_bir_lowering=False, debug=False,
                   num_devices=N_CORES)
    n_el = rows_per_core * D
    p_dram = nc.dram_tensor("p_in", [n_el], F16, kind="ExternalInput")
    t_dram = nc.dram_tensor("t_in", [n_el], BF16, kind="ExternalInput")
    o_dram = nc.dram_tensor("out_sums", [1, 2 * CHUNK], F32,
                            kind="ExternalOutput")

    plan = _iter_plan(rows_per_core)

    def n_chunks(fr):
        return (fr + CHUNK - 1) // CHUNK
    total_mm = sum(n_chunks(rr * D) for _, rr in plan)

    with tile.TileContext(nc) as tc, ExitStack() as ctx:
        const = ctx.enter_context(tc.tile_pool(name="const", bufs=1))
        io = ctx.enter_context(tc.tile_pool(name="io", bufs=4))
        work = ctx.enter_context(tc.tile_pool(name="work", bufs=3))
        psum = ctx.enter_context(
            tc.tile_pool(name="psum", bufs=1, space="PSUM"))

        ones = const.tile([128, 1], BF16)
        nc.vector.memset(ones[:], 1.0)
        bias_1eps = const.tile([128, 1], F32)
        nc.gpsimd.memset(bias_1eps[:], 1.0 + EPS)
        bias_eps = const.tile([128, 1], F32)
        nc.gpsimd.memset(bias_eps[:], EPS)
        bias_A = {}
        for (c0, c1, g, Dh, aA) in A_RANGES:
            w = 1.0 / (P * Dh)
            bt = const.tile([128, 1], F32, tag=f"lnwA{c0}")
            nc.gpsimd.memset(bt[:], math.log(w * aA))
            bias_A[c0] = bt

        pu_v = psum.tile([1, CHUNK], F32)
        pu_f2 = psum.tile([1, CHUNK], F32)

        off = 0
        mm_idx = 0
        for (npart, rr) in plan:
            fr = rr * D
            n = npart * fr
            pt = io.tile([npart, fr], F16, tag="pt")
            tt = io.tile([npart, fr], BF16, tag="tt")
            nc.sync.dma_start(
                out=pt[:],
                in_=p_dram[off:off + n].rearrange("(a b) -> a b", a=npart))
            nc.gpsimd.dma_start(
                out=tt[:],
                in_=t_dram[off:off + n].rearrange("(a b) -> a b", a=npart))

            # paired tiles: L = [l1p | lp], AB = [B | A] -- lets f1 and f2
            # run as ONE double-width TT (fewer per-instruction overheads)
            L = work.tile([npart, 2, fr], BF16, tag="L")
            AB = work.tile([npart, 2, fr], BF16, tag="AB")

            # exact ln((1+eps) - p) on ACT; fp16 p upcast to fp32 inside
            nc.scalar.activation(L[:, 0:1, :], pt[:], AF.Ln,
                                 bias=bias_1eps[0:npart, :], scale=-1.0)

            L4 = L[:].rearrange("p two (r d) -> p two r d", d=D)
            A4 = AB[:].rearrange("p two (r d) -> p two r d", d=D)
            B4i = AB[:].bitcast(I16).rearrange("p two (r d) -> p two r d",
                                               d=D)
            P4 = pt[:].rearrange("p (r d) -> p r d", d=D)
            P4i = pt[:].bitcast(I16).rearrange("p (r d) -> p r d", d=D)
            # lp: exact Ln on ACT for [0, LP_SPLIT), fastlog TS above
            nc.scalar.activation(L4[:, 1:2, :, 0:LP_SPLIT],
                                 P4[:, :, 0:LP_SPLIT],
                                 AF.Ln, bias=bias_eps[0:npart, :], scale=1.0)
            nc.vector.tensor_scalar(out=L4[:, 1:2, :, LP_SPLIT:D],
                                    in0=P4i[:, :, LP_SPLIT:D],
                                    scalar1=FL_K, scalar2=FL_C,
                                    op0=ALU.mult, op1=ALU.add)
            for (c0, c1, g, Dh, aA) in A_RANGES:
                nc.scalar.activation(A4[:, 1:2, :, c0:c1],
                                     L4[:, 0:1, :, c0:c1],
                                     AF.Exp, bias=bias_A[c0][0:npart, :],
                                     scale=g)
            for (c0, c1, g, Dh) in RANGES:
                nc.vector.tensor_scalar(out=B4i[:, 0:1, :, c0:c1],
                                        in0=P4i[:, :, c0:c1],
                                        scalar1=g / 8.0, scalar2=_cB(g, Dh),
                                        op0=ALU.mult, op1=ALU.add)

            # products in place over dead inputs: [f2|f1] -> L, d -> AB[0],
            # v -> AB[1].  (Same-index elementwise in-place is safe on DVE;
            # the WAR on L vs the A-Exp reads is serialized by the tile
            # dependency tracker.  Pool/gpsimd measured net-negative.)
            nc.vector.tensor_tensor(out=L[:], in0=AB[:], in1=L[:],
                                    op=ALU.mult)
            nc.vector.tensor_tensor(out=AB[:, 0:1, :], in0=L[:, 1:2, :],
                                    in1=L[:, 0:1, :], op=ALU.subtract)
            nc.vector.tensor_tensor(out=AB[:, 1:2, :], in0=tt[:],
                                    in1=AB[:, 0:1, :], op=ALU.mult)

            for c in range(0, fr, CHUNK):
                cw = min(CHUNK, fr - c)
                first = mm_idx == 0
                last = mm_idx == total_mm - 1
                nc.tensor.matmul(pu_v[0:1, 0:cw], ones[0:npart, 0:1],
                                 AB[:, 1:2, c:c + cw], start=first, stop=last)
                nc.tensor.matmul(pu_f2[0:1, 0:cw], ones[0:npart, 0:1],
                                 L[:, 0:1, c:c + cw], start=first, stop=last)
                mm_idx += 1
            off += n

        out_sb = const.tile([1, 2 * CHUNK], F32)
        nc.vector.tensor_copy(out_sb[0:1, 0:CHUNK], pu_v[0:1, :])
        nc.vector.tensor_copy(out_sb[0:1, CHUNK:2 * CHUNK], pu_f2[0:1, :])
        nc.sync.dma_start(out=o_dram[:], in_=out_sb[:])

    nc.compile()
    return nc


_NC = None


def _get_nc():
    global _NC
    if _NC is None:
        _NC = build_program(RPC)
    return _NC


def _combine(results):
    total = 0.0
    for res in results:
        out = np.asarray(res["out_sums"], dtype=np.float64).reshape(-1)
        total += out.sum()
    return np.float32(-total)


def kernel(predictions, targets):
    nc = _get_nc()
    p_flat = np.ascontiguousarray(predictions, dtype=np.float32).reshape(-1)
    t_flat = np.ascontiguousarray(targets, dtype=np.float32).reshape(-1)
    p16 = p_flat.astype(np.float16)
    t16 = t_flat.astype(ml_dtypes.bfloat16)
    spc = RPC * D
    in_maps = [
        {"p_in": p16[k * spc:(k + 1) * spc],
         "t_in": t16[k * spc:(k + 1) * spc]}
        for k in range(N_CORES)
    ]
    trace = bool(int(os.environ.get("KERNEL_TRACE", "0")))
    kw = {}
    if trace:
        try:
            import trace_support
            trace_support.install()
            tdir = os.environ.get("KERNEL_TRACE_DIR")
            if tdir:
                os.makedirs(tdir, exist_ok=True)
                kw["tmpdir"] = tdir
        except Exception as e:  # tracing is dev-only; never block the run
            print(f"trace support unavailable: {e}")
            trace = False
    r = run_bass_kernel_spmd(nc, in_maps, list(range(N_CORES)), trace=trace, **kw)
    if trace and r.exec_time_ns is not None:
        print(f"HW exec time: {r.exec_time_ns} ns")
    return _combine(r.results)


# revision 15
# speedup vs baseline: 1.0072x; 1.0072x over previous
"""Focal-loss kernel for Trainium2 (Bass/Tile), 8-core data-parallel.

Computes, for fp32 inputs predictions/targets of shape (32, 8400, 720):

    total = sum over 5 heads of
        sum_b mean_{p,d}( -(t*(1-pc)^g*ln(pc) + (1-t)*pc^g*ln(1-pc)) )

with pc = clip(p, 1e-7, 1-1e-7), head splits (160,160,160,160,80) and
gammas (2.5, 2.5, 2.0, 2.0, 3.0).

Final design (HW-measured engine rates drive the work split; both
ACT and DVE land ~91-96% busy at the joint equilibrium):
  - Host pre-casts p to fp16 and t to bf16, halving HBM traffic
    (96.8MB/core, ~280us at ~347GB/s/core).
  - ACT (~1 el/ns/partition): l1p = Ln((1+eps) - p) [exact, fused
    subtract], lp = Ln(p + eps) on channels [0, LP_SPLIT), and
    A = w*aA*(1-p)^g = Exp(g*l1p + ln(w*aA)).
  - DVE tensor_scalar (~2.7-3.6 el/ns): bit-trick fast math off the
    fp16 bit pattern of p:
       lp[LP_SPLIT:] = bits(p)*ln2/1024 - 15*ln2   (fastlog; chord
                                                    error folded in aA)
       bitsB = bits(p)*(g/8) + cB  -> bf16 bits of  B = w*p^g
    The log2/exp2 chord errors are deterministic functions of p; the
    per-range constants aA/cs are root-solved offline on the full fp16
    grid so E[sum] matches the fp64 reference (numpy sim rel err ~4e-7;
    measured on HW 2e-4; the gate is 2e-2).
  - DVE tensor_tensor (~1.8 el/ns, 2x mode): f1 = A*lp, f2 = B*l1p,
    d = f1 - f2 (overwrites A), v = t*d (overwrites B).
    (Pool/gpsimd tensor ops measured net-negative: 0.42 el/ns and they
    slam SBUF, degrading concurrent DVE far more than they contribute.)
  - PE: ones-matmul column sums of v and f2 into PSUM (2 streams).
  - Host: total = -(sum(v) + sum(f2)) over cores in fp64.
  - Small tiles (7 rows/partition) + io bufs=3 keep DMA ahead and cut
    the cross-engine SBUF contention tax (measured 649->546us vs rr=8).

Sharding: rows (b*p flattened: 268800 rows of 720 channels) split
contiguously across 8 cores, 33600 rows each.
"""

import math
import os
from contextlib import ExitStack

import numpy as np
import ml_dtypes

from concourse import bacc, mybir, tile
from concourse.bass_utils import run_bass_kernel_spmd

# Problem constants (hardcoded per harness contract).
B, P, D = 32, 8400, 720
N_CORES = 8
ROWS = B * P                 # 268800
RPC = ROWS // N_CORES        # 33600 rows per core
EPS = 1e-7
LN2 = math.log(2.0)

F32 = mybir.dt.float32
F16 = mybir.dt.float16
BF16 = mybir.dt.bfloat16
I16 = mybir.dt.int16
AF = mybir.ActivationFunctionType
ALU = mybir.AluOpType

# fastlog: ln(p) ~= bits_f16(p)*FL_K + FL_C
FL_K = LN2 / 1024.0
FL_C = -15.0 * LN2

# (c0, c1, gamma, head_width): contiguous channel ranges with constant
# (g, w).  CAL[(g, Dh)] = (aA, cs): offline-calibrated so the expected
# f1/f2 sums over the fp16 grid match the fp64 reference (see module doc).
RANGES = [
    (0, 320, 2.5, 160),
    (320, 640, 2.0, 160),
    (640, 720, 3.0, 80),
]
CAL = {
    (2.5, 160): (0.980244, 4.5250),
    (2.0, 160): (0.978827, 2.3365),
    (3.0, 80): (0.981308, 6.7113),
}
# lp is exact ACT-Ln on channels [0, LP_SPLIT) and DVE fastlog on
# [LP_SPLIT, 720) -- balances the two engines.  A's aA constant differs
# by lp method (the fastlog chord bias is folded into aA).
LP_SPLIT = 560
A_RANGES = [
    (0, 320, 2.5, 160, 0.999973),
    (320, 560, 2.0, 160, 0.999999),
    (560, 640, 2.0, 160, 0.978827),
    (640, 720, 3.0, 80, 0.981308),
]

R_MAIN = 7        # rows per partition per main-loop tile
CHUNK = 504       # matmul moving free-dim chunk (<=512); 7*720 = 10 chunks

_ACT_SET = "natural_log_exp_and_others"
_act_tables_patched = False


def _pin_act_table_set():
    """Make Ln/Exp resolve only to the one table set containing both, so
    the table-load pass emits a single load instead of thrashing."""
    global _act_tables_patched
    if _act_tables_patched:
        return
    orig = bacc.get_activation_tables

    def patched(arch):
        tables = orig(arch)
        pinned = {AF.Ln, AF.Exp}
        assert pinned <= tables[_ACT_SET], tables[_ACT_SET]
        return {
            name: (funcs if name == _ACT_SET else funcs - pinned)
            for name, funcs in tables.items()
        }

    bacc.get_activation_tables = patched
    _act_tables_patched = True


def _iter_plan(rows):
    """Split `rows` into (npart, rows_per_partition) tiles."""
    plan = []
    r = rows
    # two small pipeline-fill tiles first: the first TT chain starts after
    # ~2 rows of DMA+ACT instead of R_MAIN rows
    for rr0 in (2, 4):
        if r >= 128 * (rr0 + R_MAIN):
            plan.append((128, rr0))
            r -= 128 * rr0
    while r >= 128 * R_MAIN:
        plan.append((128, R_MAIN))
        r -= 128 * R_MAIN
    if r >= 128:
        plan.append((128, r // 128))
        r -= 128 * (r // 128)
    if r:
        assert r % 64 == 0, r
        plan.append((r, 1))
    return plan


def _cB(g, Dh):
    w = 1.0 / (P * Dh)
    aA, cs = CAL[(g, Dh)]
    return 16256.0 + 128.0 * (math.log2(w) - 15.0 * g) + cs


def build_program(rows_per_core=RPC):
    _pin_act_table_set()
    nc = bacc.Bacc("TRN2", target_bir_lowering=False, debug=False,
                   num_devices=N_CORES)
    n_el = rows_per_core * D
    p_dram = nc.dram_tensor("p_in", [n_el], F16, kind="ExternalInput")
    t_dram = nc.dram_tensor("t_in", [n_el], BF16, kind="ExternalInput")
    o_dram = nc.dram_tensor("out_sums", [1, 2 * CHUNK], F32,
                            kind="ExternalOutput")

    plan = _iter_plan(rows_per_core)

    def n_chunks(fr):
        return (fr + CHUNK - 1) // CHUNK
    total_mm = sum(n_chunks(rr * D) for _, rr in plan)

    with tile.TileContext(nc) as tc, ExitStack() as ctx:
        const = ctx.enter_context(tc.tile_pool(name="const", bufs=1))
        io = ctx.enter_context(tc.tile_pool(name="io", bufs=4))
        work = ctx.enter_context(tc.tile_pool(name="work", bufs=3))
        psum = ctx.enter_context(
            tc.tile_pool(name="psum", bufs=1, space="PSUM"))

        ones = const.tile([128, 1], BF16)
        nc.vector.memset(ones[:], 1.0)
        bias_1eps = const.tile([128, 1], F32)
        nc.gpsimd.memset(bias_1eps[:], 1.0 + EPS)
        bias_eps = const.tile([128, 1], F32)
        nc.gpsimd.memset(bias_eps[:], EPS)
        bias_A = {}
        for (c0, c1, g, Dh, aA) in A_RANGES:
            w = 1.0 / (P * Dh)
            bt = const.tile([128, 1], F32, tag=f"lnwA{c0}")
            nc.gpsimd.memset(bt[:], math.log(w * aA))
            bias_A[c0] = bt

        pu_v = psum.tile([1, CHUNK], F32)
        pu_f2 = psum.tile([1, CHUNK], F32)

        off = 0
        mm_idx = 0
        for (npart, rr) in plan:
            fr = rr * D
            n = npart * fr
            pt = io.tile([npart, fr], F16, tag="pt")
            tt = io.tile([npart, fr], BF16, tag="tt")
            nc.sync.dma_start(
                out=pt[:],
                in_=p_dram[off:off + n].rearrange("(a b) -> a b", a=npart))
            nc.gpsimd.dma_start(
                out=tt[:],
                in_=t_dram[off:off + n].rearrange("(a b) -> a b", a=npart))

            lp = work.tile([npart, fr], BF16, tag="lp")
            l1p = work.tile([npart, fr], BF16, tag="l1p")
            A = work.tile([npart, fr], BF16, tag="A")
            Bt = work.tile([npart, fr], BF16, tag="B")

            # exact ln((1+eps) - p) on ACT; fp16 p upcast to fp32 inside
            nc.scalar.activation(l1p[:], pt[:], AF.Ln,
                                 bias=bias_1eps[0:npart, :], scale=-1.0)

            L4 = l1p[:].rearrange("p (r d) -> p r d", d=D)
            A4 = A[:].rearrange("p (r d) -> p r d", d=D)
            LP4 = lp[:].rearrange("p (r d) -> p r d", d=D)
            P4 = pt[:].rearrange("p (r d) -> p r d", d=D)
            P4i = pt[:].bitcast(I16).rearrange("p (r d) -> p r d", d=D)
            B4i = Bt[:].bitcast(I16).rearrange("p (r d) -> p r d", d=D)
            # lp: exact Ln on ACT for [0, LP_SPLIT), fastlog TS above
            nc.scalar.activation(LP4[:, :, 0:LP_SPLIT], P4[:, :, 0:LP_SPLIT],
                                 AF.Ln, bias=bias_eps[0:npart, :], scale=1.0)
            nc.vector.tensor_scalar(out=LP4[:, :, LP_SPLIT:D],
                                    in0=P4i[:, :, LP_SPLIT:D],
                                    scalar1=FL_K, scalar2=FL_C,
                                    op0=ALU.mult, op1=ALU.add)
            for (c0, c1, g, Dh, aA) in A_RANGES:
                nc.scalar.activation(A4[:, :, c0:c1], L4[:, :, c0:c1],
                                     AF.Exp, bias=bias_A[c0][0:npart, :],
                                     scale=g)
            for (c0, c1, g, Dh) in RANGES:
                nc.vector.tensor_scalar(out=B4i[:, :, c0:c1],
                                        in0=P4i[:, :, c0:c1],
                                        scalar1=g / 8.0, scalar2=_cB(g, Dh),
                                        op0=ALU.mult, op1=ALU.add)

            # products run in place over their dead inputs: f1 -> lp,
            # f2 -> l1p, d -> A, v -> B.  (Same-index elementwise in-place
            # is safe on DVE; the WAR on l1p vs the A-Exp reads is
            # serialized by the tile dependency tracker.  Pool/gpsimd
            # measured net-negative for any of these passes.)
            nc.vector.tensor_tensor(out=lp[:], in0=A[:], in1=lp[:],
                                    op=ALU.mult)
            nc.vector.tensor_tensor(out=l1p[:], in0=Bt[:], in1=l1p[:],
                                    op=ALU.mult)
            nc.vector.tensor_tensor(out=A[:], in0=lp[:], in1=l1p[:],
                                    op=ALU.subtract)
            nc.vector.tensor_tensor(out=Bt[:], in0=tt[:], in1=A[:],
                                    op=ALU.mult)

            for c in range(0, fr, CHUNK):
                cw = min(CHUNK, fr - c)
                first = mm_idx == 0
                last = mm_idx == total_mm - 1
                nc.tensor.matmul(pu_v[0:1, 0:cw], ones[0:npart, 0:1],
                                 Bt[:, c:c + cw], start=first, stop=last)
                nc.tensor.matmul(pu_f2[0:1, 0:cw], ones[0:npart, 0:1],
                                 l1p[:, c:c + cw], start=first, stop=last)
                mm_idx += 1
            off += n

        out_sb = const.tile([1, 2 * CHUNK], F32)
        nc.vector.tensor_copy(out_sb[0:1, 0:CHUNK], pu_v[0:1, :])
        nc.vector.tensor_copy(out_sb[0:1, CHUNK:2 * CHUNK], pu_f2[0:1, :])
        nc.sync.dma_start(out=o_dram[:], in_=out_sb[:])

    nc.compile()
    return nc


_NC = None


def _get_nc():
    global _NC
    if _NC is None:
        _NC = build_program(RPC)
    return _NC


def _combine(results):
    total = 0.0
    for res in results:
        out = np.asarray(res["out_sums"], dtype=np.float64).reshape(-1)
        total += out.sum()
    return np.float32(-total)


def kernel(predictions, targets):
    nc = _get_nc()
    p_flat = np.ascontiguousarray(predictions, dtype=np.float32).reshape(-1)
    t_flat = np.ascontiguousarray(targets, dtype=np.float32).reshape(-1)
    p16 = p_flat.astype(np.float16)
    t16 = t_flat.astype(ml_dtypes.bfloat16)
    spc = RPC * D
    in_maps = [
        {"p_in": p16[k * spc:(k + 1) * spc],
         "t_in": t16[k * spc:(k + 1) * spc]}
        for k in range(N_CORES)
    ]
    trace = bool(int(os.environ.get("KERNEL_TRACE", "0")))
    kw = {}
    if trace:
        try:
            import trace_support
            trace_support.install()
            tdir = os.environ.get("KERNEL_TRACE_DIR")
            if tdir:
                os.makedirs(tdir, exist_ok=True)
                kw["tmpdir"] = tdir
        except Exception as e:  # tracing is dev-only; never block the run
            print(f"trace support unavailable: {e}")
            trace = False
    r = run_bass_kernel_spmd(nc, in_maps, list(range(N_CORES)), trace=trace, **kw)
    if trace and r.exec_time_ns is not None:
        print(f"HW exec time: {r.exec_time_ns} ns")
    return _combine(r.results)


# revision 16
# speedup vs baseline: 1.0155x; 1.0083x over previous
"""Focal-loss kernel for Trainium2 (Bass/Tile), 8-core data-parallel.

Computes, for fp32 inputs predictions/targets of shape (32, 8400, 720):

    total = sum over 5 heads of
        sum_b mean_{p,d}( -(t*(1-pc)^g*ln(pc) + (1-t)*pc^g*ln(1-pc)) )

with pc = clip(p, 1e-7, 1-1e-7), head splits (160,160,160,160,80) and
gammas (2.5, 2.5, 2.0, 2.0, 3.0).

Final design (HW-measured engine rates drive the work split; both
ACT and DVE land ~91-96% busy at the joint equilibrium):
  - Host pre-casts p to fp16 and t to bf16, halving HBM traffic
    (96.8MB/core, ~280us at ~347GB/s/core).
  - ACT (~1 el/ns/partition): l1p = Ln((1+eps) - p) [exact, fused
    subtract], lp = Ln(p + eps) on channels [0, LP_SPLIT), and
    A = w*aA*(1-p)^g = Exp(g*l1p + ln(w*aA)).
  - DVE tensor_scalar (~2.7-3.6 el/ns): bit-trick fast math off the
    fp16 bit pattern of p:
       lp[LP_SPLIT:] = bits(p)*ln2/1024 - 15*ln2   (fastlog; chord
                                                    error folded in aA)
       bitsB = bits(p)*(g/8) + cB  -> bf16 bits of  B = w*p^g
    The log2/exp2 chord errors are deterministic functions of p; the
    per-range constants aA/cs are root-solved offline on the full fp16
    grid so E[sum] matches the fp64 reference (numpy sim rel err ~4e-7;
    measured on HW 2e-4; the gate is 2e-2).
  - DVE tensor_tensor (~1.8 el/ns, 2x mode): f1 = A*lp, f2 = B*l1p,
    d = f1 - f2 (overwrites A), v = t*d (overwrites B).
    (Pool/gpsimd tensor ops measured net-negative: 0.42 el/ns and they
    slam SBUF, degrading concurrent DVE far more than they contribute.)
  - PE: ones-matmul column sums of v and f2 into PSUM (2 streams).
  - Host: total = -(sum(v) + sum(f2)) over cores in fp64.
  - Small tiles (7 rows/partition) + io bufs=3 keep DMA ahead and cut
    the cross-engine SBUF contention tax (measured 649->546us vs rr=8).

Sharding: rows (b*p flattened: 268800 rows of 720 channels) split
contiguously across 8 cores, 33600 rows each.
"""

import math
import os
from contextlib import ExitStack

import numpy as np
import ml_dtypes

from concourse import bacc, mybir, tile
from concourse.bass_utils import run_bass_kernel_spmd

# Problem constants (hardcoded per harness contract).
B, P, D = 32, 8400, 720
N_CORES = 8
ROWS = B * P                 # 268800
RPC = ROWS // N_CORES        # 33600 rows per core
EPS = 1e-7
LN2 = math.log(2.0)

F32 = mybir.dt.float32
F16 = mybir.dt.float16
BF16 = mybir.dt.bfloat16
I16 = mybir.dt.int16
AF = mybir.ActivationFunctionType
ALU = mybir.AluOpType

# fastlog: ln(p) ~= bits_f16(p)*FL_K + FL_C
FL_K = LN2 / 1024.0
FL_C = -15.0 * LN2

# (c0, c1, gamma, head_width): contiguous channel ranges with constant
# (g, w).  CAL[(g, Dh)] = (aA, cs): offline-calibrated so the expected
# f1/f2 sums over the fp16 grid match the fp64 reference (see module doc).
RANGES = [
    (0, 320, 2.5, 160),
    (320, 640, 2.0, 160),
    (640, 720, 3.0, 80),
]
CAL = {
    (2.5, 160): (0.980244, 4.5250),
    (2.0, 160): (0.978827, 2.3365),
    (3.0, 80): (0.981308, 6.7113),
}
# lp is exact ACT-Ln on channels [0, LP_SPLIT) and DVE fastlog on
# [LP_SPLIT, 720) -- balances the two engines.  A's aA constant differs
# by lp method (the fastlog chord bias is folded into aA).
LP_SPLIT = 560
A_RANGES = [
    (0, 320, 2.5, 160, 0.999973),
    (320, 560, 2.0, 160, 0.999999),
    (560, 640, 2.0, 160, 0.978827),
    (640, 720, 3.0, 80, 0.981308),
]

R_MAIN = 8        # rows per partition per main-loop tile
CHUNK = 480       # matmul moving free-dim chunk (<=512); 8*720 = 12 chunks

_ACT_SET = "natural_log_exp_and_others"
_act_tables_patched = False


def _pin_act_table_set():
    """Make Ln/Exp resolve only to the one table set containing both, so
    the table-load pass emits a single load instead of thrashing."""
    global _act_tables_patched
    if _act_tables_patched:
        return
    orig = bacc.get_activation_tables

    def patched(arch):
        tables = orig(arch)
        pinned = {AF.Ln, AF.Exp}
        assert pinned <= tables[_ACT_SET], tables[_ACT_SET]
        return {
            name: (funcs if name == _ACT_SET else funcs - pinned)
            for name, funcs in tables.items()
        }

    bacc.get_activation_tables = patched
    _act_tables_patched = True


def _iter_plan(rows):
    """Split `rows` into (npart, rows_per_partition) tiles."""
    plan = []
    r = rows
    # two small pipeline-fill tiles first: the first TT chain starts after
    # ~2 rows of DMA+ACT instead of R_MAIN rows
    for rr0 in (2, 4):
        if r >= 128 * (rr0 + R_MAIN):
            plan.append((128, rr0))
            r -= 128 * rr0
    while r >= 128 * R_MAIN:
        plan.append((128, R_MAIN))
        r -= 128 * R_MAIN
    if r >= 128:
        plan.append((128, r // 128))
        r -= 128 * (r // 128)
    if r:
        assert r % 64 == 0, r
        plan.append((r, 1))
    return plan


def _cB(g, Dh):
    w = 1.0 / (P * Dh)
    aA, cs = CAL[(g, Dh)]
    return 16256.0 + 128.0 * (math.log2(w) - 15.0 * g) + cs


def build_program(rows_per_core=RPC):
    _pin_act_table_set()
    nc = bacc.Bacc("TRN2", target_bir_lowering=False, debug=False,
                   num_devices=N_CORES)
    n_el = rows_per_core * D
    p_dram = nc.dram_tensor("p_in", [n_el], F16, kind="ExternalInput")
    t_dram = nc.dram_tensor("t_in", [n_el], BF16, kind="ExternalInput")
    o_dram = nc.dram_tensor("out_sums", [1, 2 * CHUNK], F32,
                            kind="ExternalOutput")

    plan = _iter_plan(rows_per_core)

    def n_chunks(fr):
        return (fr + CHUNK - 1) // CHUNK
    total_mm = sum(n_chunks(rr * D) for _, rr in plan)

    with tile.TileContext(nc) as tc, ExitStack() as ctx:
        const = ctx.enter_context(tc.tile_pool(name="const", bufs=1))
        io = ctx.enter_context(tc.tile_pool(name="io", bufs=4))
        work = ctx.enter_context(tc.tile_pool(name="work", bufs=2))
        psum = ctx.enter_context(
            tc.tile_pool(name="psum", bufs=1, space="PSUM"))

        ones = const.tile([128, 1], BF16)
        nc.vector.memset(ones[:], 1.0)
        bias_1eps = const.tile([128, 1], F32)
        nc.gpsimd.memset(bias_1eps[:], 1.0 + EPS)
        bias_eps = const.tile([128, 1], F32)
        nc.gpsimd.memset(bias_eps[:], EPS)
        bias_A = {}
        for (c0, c1, g, Dh, aA) in A_RANGES:
            w = 1.0 / (P * Dh)
            bt = const.tile([128, 1], F32, tag=f"lnwA{c0}")
            nc.gpsimd.memset(bt[:], math.log(w * aA))
            bias_A[c0] = bt

        pu_v = psum.tile([1, CHUNK], F32)
        pu_f2 = psum.tile([1, CHUNK], F32)

        off = 0
        mm_idx = 0
        for (npart, rr) in plan:
            fr = rr * D
            n = npart * fr
            pt = io.tile([npart, fr], F16, tag="pt")
            tt = io.tile([npart, fr], BF16, tag="tt")
            nc.sync.dma_start(
                out=pt[:],
                in_=p_dram[off:off + n].rearrange("(a b) -> a b", a=npart))
            nc.gpsimd.dma_start(
                out=tt[:],
                in_=t_dram[off:off + n].rearrange("(a b) -> a b", a=npart))

            lp = work.tile([npart, fr], BF16, tag="lp")
            l1p = work.tile([npart, fr], BF16, tag="l1p")
            A = work.tile([npart, fr], BF16, tag="A")
            Bt = work.tile([npart, fr], BF16, tag="B")

            # exact ln((1+eps) - p) on ACT; fp16 p upcast to fp32 inside
            nc.scalar.activation(l1p[:], pt[:], AF.Ln,
                                 bias=bias_1eps[0:npart, :], scale=-1.0)

            L4 = l1p[:].rearrange("p (r d) -> p r d", d=D)
            A4 = A[:].rearrange("p (r d) -> p r d", d=D)
            LP4 = lp[:].rearrange("p (r d) -> p r d", d=D)
            P4 = pt[:].rearrange("p (r d) -> p r d", d=D)
            P4i = pt[:].bitcast(I16).rearrange("p (r d) -> p r d", d=D)
            B4i = Bt[:].bitcast(I16).rearrange("p (r d) -> p r d", d=D)
            # lp: exact Ln on ACT for [0, LP_SPLIT), fastlog TS above
            nc.scalar.activation(LP4[:, :, 0:LP_SPLIT], P4[:, :, 0:LP_SPLIT],
                                 AF.Ln, bias=bias_eps[0:npart, :], scale=1.0)
            nc.vector.tensor_scalar(out=LP4[:, :, LP_SPLIT:D],
                                    in0=P4i[:, :, LP_SPLIT:D],
                                    scalar1=FL_K, scalar2=FL_C,
                                    op0=ALU.mult, op1=ALU.add)
            for (c0, c1, g, Dh, aA) in A_RANGES:
                nc.scalar.activation(A4[:, :, c0:c1], L4[:, :, c0:c1],
                                     AF.Exp, bias=bias_A[c0][0:npart, :],
                                     scale=g)
            for (c0, c1, g, Dh) in RANGES:
                nc.vector.tensor_scalar(out=B4i[:, :, c0:c1],
                                        in0=P4i[:, :, c0:c1],
                                        scalar1=g / 8.0, scalar2=_cB(g, Dh),
                                        op0=ALU.mult, op1=ALU.add)

            # products run in place over their dead inputs: f1 -> lp,
            # f2 -> l1p, d -> A, v -> B.  (Same-index elementwise in-place
            # is safe on DVE; the WAR on l1p vs the A-Exp reads is
            # serialized by the tile dependency tracker.  Pool/gpsimd
            # measured net-negative for any of these passes.)
            nc.vector.tensor_tensor(out=lp[:], in0=A[:], in1=lp[:],
                                    op=ALU.mult)
            nc.vector.tensor_tensor(out=l1p[:], in0=Bt[:], in1=l1p[:],
                                    op=ALU.mult)
            nc.vector.tensor_tensor(out=A[:], in0=lp[:], in1=l1p[:],
                                    op=ALU.subtract)
            nc.vector.tensor_tensor(out=Bt[:], in0=tt[:], in1=A[:],
                                    op=ALU.mult)

            for c in range(0, fr, CHUNK):
                cw = min(CHUNK, fr - c)
                first = mm_idx == 0
                last = mm_idx == total_mm - 1
                nc.tensor.matmul(pu_v[0:1, 0:cw], ones[0:npart, 0:1],
                                 Bt[:, c:c + cw], start=first, stop=last)
                nc.tensor.matmul(pu_f2[0:1, 0:cw], ones[0:npart, 0:1],
                                 l1p[:, c:c + cw], start=first, stop=last)
                mm_idx += 1
            off += n

        out_sb = const.tile([1, 2 * CHUNK], F32)
        nc.vector.tensor_copy(out_sb[0:1, 0:CHUNK], pu_v[0:1, :])
        nc.vector.tensor_copy(out_sb[0:1, CHUNK:2 * CHUNK], pu_f2[0:1, :])
        nc.sync.dma_start(out=o_dram[:], in_=out_sb[:])

    nc.compile()
    return nc


_NC = None


def _get_nc():
    global _NC
    if _NC is None:
        _NC = build_program(RPC)
    return _NC


def _combine(results):
    total = 0.0
    for res in results:
        out = np.asarray(res["out_sums"], dtype=np.float64).reshape(-1)
        total += out.sum()
    return np.float32(-total)


def kernel(predictions, targets):
    nc = _get_nc()
    p_flat = np.ascontiguousarray(predictions, dtype=np.float32).reshape(-1)
    t_flat = np.ascontiguousarray(targets, dtype=np.float32).reshape(-1)
    p16 = p_flat.astype(np.float16)
    t16 = t_flat.astype(ml_dtypes.bfloat16)
    spc = RPC * D
    in_maps = [
        {"p_in": p16[k * spc:(k + 1) * spc],
         "t_in": t16[k * spc:(k + 1) * spc]}
        for k in range(N_CORES)
    ]
    trace = bool(int(os.environ.get("KERNEL_TRACE", "0")))
    kw = {}
    if trace:
        try:
            import trace_support
            trace_support.install()
            tdir = os.environ.get("KERNEL_TRACE_DIR")
            if tdir:
                os.makedirs(tdir, exist_ok=True)
                kw["tmpdir"] = tdir
        except Exception as e:  # tracing is dev-only; never block the run
            print(f"trace support unavailable: {e}")
            trace = False
    r = run_bass_kernel_spmd(nc, in_maps, list(range(N_CORES)), trace=trace, **kw)
    if trace and r.exec_time_ns is not None:
        print(f"HW exec time: {r.exec_time_ns} ns")
    return _combine(r.results)


# revision 17
# speedup vs baseline: 1.0229x; 1.0072x over previous
"""Focal-loss kernel for Trainium2 (Bass/Tile), 8-core data-parallel.

Computes, for fp32 inputs predictions/targets of shape (32, 8400, 720):

    total = sum over 5 heads of
        sum_b mean_{p,d}( -(t*(1-pc)^g*ln(pc) + (1-t)*pc^g*ln(1-pc)) )

with pc = clip(p, 1e-7, 1-1e-7), head splits (160,160,160,160,80) and
gammas (2.5, 2.5, 2.0, 2.0, 3.0).

Final design (HW-measured engine rates drive the work split; both
ACT and DVE land ~91-96% busy at the joint equilibrium):
  - Host pre-casts p to fp16 and t to bf16, halving HBM traffic
    (96.8MB/core, ~280us at ~347GB/s/core).
  - ACT (~1 el/ns/partition): l1p = Ln((1+eps) - p) [exact, fused
    subtract], lp = Ln(p + eps) on channels [0, LP_SPLIT), and
    A = w*aA*(1-p)^g = Exp(g*l1p + ln(w*aA)).
  - DVE tensor_scalar (~2.7-3.6 el/ns): bit-trick fast math off the
    fp16 bit pattern of p:
       lp[LP_SPLIT:] = bits(p)*ln2/1024 - 15*ln2   (fastlog; chord
                                                    error folded in aA)
       bitsB = bits(p)*(g/8) + cB  -> bf16 bits of  B = w*p^g
    The log2/exp2 chord errors are deterministic functions of p; the
    per-range constants aA/cs are root-solved offline on the full fp16
    grid so E[sum] matches the fp64 reference (numpy sim rel err ~4e-7;
    measured on HW 2e-4; the gate is 2e-2).
  - DVE tensor_tensor (~1.8 el/ns, 2x mode): f1 = A*lp, f2 = B*l1p,
    d = f1 - f2 (overwrites A), v = t*d (overwrites B).
    (Pool/gpsimd tensor ops measured net-negative: 0.42 el/ns and they
    slam SBUF, degrading concurrent DVE far more than they contribute.)
  - PE: ones-matmul column sums of v and f2 into PSUM (2 streams).
  - Host: total = -(sum(v) + sum(f2)) over cores in fp64.
  - Small tiles (7 rows/partition) + io bufs=3 keep DMA ahead and cut
    the cross-engine SBUF contention tax (measured 649->546us vs rr=8).

Sharding: rows (b*p flattened: 268800 rows of 720 channels) split
contiguously across 8 cores, 33600 rows each.
"""

import math
import os
from contextlib import ExitStack

import numpy as np
import ml_dtypes

from concourse import bacc, mybir, tile
from concourse.bass_utils import run_bass_kernel_spmd

# Problem constants (hardcoded per harness contract).
B, P, D = 32, 8400, 720
N_CORES = 8
ROWS = B * P                 # 268800
RPC = ROWS // N_CORES        # 33600 rows per core
EPS = 1e-7
LN2 = math.log(2.0)

F32 = mybir.dt.float32
F16 = mybir.dt.float16
BF16 = mybir.dt.bfloat16
I16 = mybir.dt.int16
AF = mybir.ActivationFunctionType
ALU = mybir.AluOpType

# fastlog: ln(p) ~= bits_f16(p)*FL_K + FL_C
FL_K = LN2 / 1024.0
FL_C = -15.0 * LN2

# (c0, c1, gamma, head_width): contiguous channel ranges with constant
# (g, w).  CAL[(g, Dh)] = (aA, cs): offline-calibrated so the expected
# f1/f2 sums over the fp16 grid match the fp64 reference (see module doc).
RANGES = [
    (0, 320, 2.5, 160),
    (320, 640, 2.0, 160),
    (640, 720, 3.0, 80),
]
CAL = {
    (2.5, 160): (0.980244, 4.5250),
    (2.0, 160): (0.978827, 2.3365),
    (3.0, 80): (0.981308, 6.7113),
}
# lp is exact ACT-Ln on channels [0, LP_SPLIT) and DVE fastlog on
# [LP_SPLIT, 720) -- balances the two engines.  A's aA constant differs
# by lp method (the fastlog chord bias is folded into aA).
LP_SPLIT = 560
A_RANGES = [
    (0, 320, 2.5, 160, 0.999973),
    (320, 560, 2.0, 160, 0.999999),
    (560, 640, 2.0, 160, 0.978827),
    (640, 720, 3.0, 80, 0.981308),
]

R_MAIN = 8        # rows per partition per main-loop tile
CHUNK = 480       # matmul moving free-dim chunk (<=512); 8*720 = 12 chunks

_ACT_SET = "natural_log_exp_and_others"
_act_tables_patched = False


def _pin_act_table_set():
    """Make Ln/Exp resolve only to the one table set containing both, so
    the table-load pass emits a single load instead of thrashing."""
    global _act_tables_patched
    if _act_tables_patched:
        return
    orig = bacc.get_activation_tables

    def patched(arch):
        tables = orig(arch)
        pinned = {AF.Ln, AF.Exp}
        assert pinned <= tables[_ACT_SET], tables[_ACT_SET]
        return {
            name: (funcs if name == _ACT_SET else funcs - pinned)
            for name, funcs in tables.items()
        }

    bacc.get_activation_tables = patched
    _act_tables_patched = True


def _iter_plan(rows):
    """Split `rows` into (npart, rows_per_partition) tiles."""
    plan = []
    r = rows
    # two small pipeline-fill tiles first: the first TT chain starts after
    # ~2 rows of DMA+ACT instead of R_MAIN rows
    for rr0 in (2, 4):
        if r >= 128 * (rr0 + R_MAIN):
            plan.append((128, rr0))
            r -= 128 * rr0
    # drain taper: descending tile sizes at the end so the final
    # matmul burst after the last TT is short
    TAIL = [(128, 6), (128, 5), (128, 3), (128, 2)]
    tailrows = sum(a * b for a, b in TAIL) + 64
    rem = r - tailrows
    if rem > 0 and rem % (128 * R_MAIN) == 0:
        plan += [(128, R_MAIN)] * (rem // (128 * R_MAIN))
        plan += TAIL
        plan.append((64, 1))
        return plan
    while r >= 128 * R_MAIN:
        plan.append((128, R_MAIN))
        r -= 128 * R_MAIN
    if r >= 128:
        plan.append((128, r // 128))
        r -= 128 * (r // 128)
    if r:
        assert r % 64 == 0, r
        plan.append((r, 1))
    return plan


def _cB(g, Dh):
    w = 1.0 / (P * Dh)
    aA, cs = CAL[(g, Dh)]
    return 16256.0 + 128.0 * (math.log2(w) - 15.0 * g) + cs


def build_program(rows_per_core=RPC):
    _pin_act_table_set()
    nc = bacc.Bacc("TRN2", target_bir_lowering=False, debug=False,
                   num_devices=N_CORES)
    n_el = rows_per_core * D
    p_dram = nc.dram_tensor("p_in", [n_el], F16, kind="ExternalInput")
    t_dram = nc.dram_tensor("t_in", [n_el], BF16, kind="ExternalInput")
    o_dram = nc.dram_tensor("out_sums", [1, 2 * CHUNK], F32,
                            kind="ExternalOutput")

    plan = _iter_plan(rows_per_core)

    def n_chunks(fr):
        return (fr + CHUNK - 1) // CHUNK
    total_mm = sum(n_chunks(rr * D) for _, rr in plan)

    with tile.TileContext(nc) as tc, ExitStack() as ctx:
        const = ctx.enter_context(tc.tile_pool(name="const", bufs=1))
        io = ctx.enter_context(tc.tile_pool(name="io", bufs=4))
        work = ctx.enter_context(tc.tile_pool(name="work", bufs=2))
        psum = ctx.enter_context(
            tc.tile_pool(name="psum", bufs=1, space="PSUM"))

        ones = const.tile([128, 1], BF16)
        nc.vector.memset(ones[:], 1.0)
        bias_1eps = const.tile([128, 1], F32)
        nc.gpsimd.memset(bias_1eps[:], 1.0 + EPS)
        bias_eps = const.tile([128, 1], F32)
        nc.gpsimd.memset(bias_eps[:], EPS)
        bias_A = {}
        for (c0, c1, g, Dh, aA) in A_RANGES:
            w = 1.0 / (P * Dh)
            bt = const.tile([128, 1], F32, tag=f"lnwA{c0}")
            nc.gpsimd.memset(bt[:], math.log(w * aA))
            bias_A[c0] = bt

        pu_v = psum.tile([1, CHUNK], F32)
        pu_f2 = psum.tile([1, CHUNK], F32)

        off = 0
        mm_idx = 0
        for (npart, rr) in plan:
            fr = rr * D
            n = npart * fr
            pt = io.tile([npart, fr], F16, tag="pt")
            tt = io.tile([npart, fr], BF16, tag="tt")
            nc.sync.dma_start(
                out=pt[:],
                in_=p_dram[off:off + n].rearrange("(a b) -> a b", a=npart))
            nc.gpsimd.dma_start(
                out=tt[:],
                in_=t_dram[off:off + n].rearrange("(a b) -> a b", a=npart))

            lp = work.tile([npart, fr], BF16, tag="lp")
            l1p = work.tile([npart, fr], BF16, tag="l1p")
            A = work.tile([npart, fr], BF16, tag="A")
            Bt = work.tile([npart, fr], BF16, tag="B")

            # exact ln((1+eps) - p) on ACT; fp16 p upcast to fp32 inside
            nc.scalar.activation(l1p[:], pt[:], AF.Ln,
                                 bias=bias_1eps[0:npart, :], scale=-1.0)

            L4 = l1p[:].rearrange("p (r d) -> p r d", d=D)
            A4 = A[:].rearrange("p (r d) -> p r d", d=D)
            LP4 = lp[:].rearrange("p (r d) -> p r d", d=D)
            P4 = pt[:].rearrange("p (r d) -> p r d", d=D)
            P4i = pt[:].bitcast(I16).rearrange("p (r d) -> p r d", d=D)
            B4i = Bt[:].bitcast(I16).rearrange("p (r d) -> p r d", d=D)
            # lp: exact Ln on ACT for [0, LP_SPLIT), fastlog TS above
            nc.scalar.activation(LP4[:, :, 0:LP_SPLIT], P4[:, :, 0:LP_SPLIT],
                                 AF.Ln, bias=bias_eps[0:npart, :], scale=1.0)
            nc.vector.tensor_scalar(out=LP4[:, :, LP_SPLIT:D],
                                    in0=P4i[:, :, LP_SPLIT:D],
                                    scalar1=FL_K, scalar2=FL_C,
                                    op0=ALU.mult, op1=ALU.add)
            for (c0, c1, g, Dh, aA) in A_RANGES:
                nc.scalar.activation(A4[:, :, c0:c1], L4[:, :, c0:c1],
                                     AF.Exp, bias=bias_A[c0][0:npart, :],
                                     scale=g)
            for (c0, c1, g, Dh) in RANGES:
                nc.vector.tensor_scalar(out=B4i[:, :, c0:c1],
                                        in0=P4i[:, :, c0:c1],
                                        scalar1=g / 8.0, scalar2=_cB(g, Dh),
                                        op0=ALU.mult, op1=ALU.add)

            # products run in place over their dead inputs: f1 -> lp,
            # f2 -> l1p, d -> A, v -> B.  (Same-index elementwise in-place
            # is safe on DVE; the WAR on l1p vs the A-Exp reads is
            # serialized by the tile dependency tracker.  Pool/gpsimd
            # measured net-negative for any of these passes.)
            nc.vector.tensor_tensor(out=lp[:], in0=A[:], in1=lp[:],
                                    op=ALU.mult)
            nc.vector.tensor_tensor(out=l1p[:], in0=Bt[:], in1=l1p[:],
                                    op=ALU.mult)
            nc.vector.tensor_tensor(out=A[:], in0=lp[:], in1=l1p[:],
                                    op=ALU.subtract)
            nc.vector.tensor_tensor(out=Bt[:], in0=tt[:], in1=A[:],
                                    op=ALU.mult)

            for c in range(0, fr, CHUNK):
                cw = min(CHUNK, fr - c)
                first = mm_idx == 0
                last = mm_idx == total_mm - 1
                nc.tensor.matmul(pu_v[0:1, 0:cw], ones[0:npart, 0:1],
                                 Bt[:, c:c + cw], start=first, stop=last)
                nc.tensor.matmul(pu_f2[0:1, 0:cw], ones[0:npart, 0:1],
                                 l1p[:, c:c + cw], start=first, stop=last)
                mm_idx += 1
            off += n

        out_sb = const.tile([1, 2 * CHUNK], F32)
        nc.vector.tensor_copy(out_sb[0:1, 0:CHUNK], pu_v[0:1, :])
        nc.vector.tensor_copy(out_sb[0:1, CHUNK:2 * CHUNK], pu_f2[0:1, :])
        nc.sync.dma_start(out=o_dram[:], in_=out_sb[:])

    nc.compile()
    return nc


_NC = None


def _get_nc():
    global _NC
    if _NC is None:
        _NC = build_program(RPC)
    return _NC


def _combine(results):
    total = 0.0
    for res in results:
        out = np.asarray(res["out_sums"], dtype=np.float64).reshape(-1)
        total += out.sum()
    return np.float32(-total)


def kernel(predictions, targets):
    nc = _get_nc()
    p_flat = np.ascontiguousarray(predictions, dtype=np.float32).reshape(-1)
    t_flat = np.ascontiguousarray(targets, dtype=np.float32).reshape(-1)
    p16 = p_flat.astype(np.float16)
    t16 = t_flat.astype(ml_dtypes.bfloat16)
    spc = RPC * D
    in_maps = [
        {"p_in": p16[k * spc:(k + 1) * spc],
         "t_in": t16[k * spc:(k + 1) * spc]}
        for k in range(N_CORES)
    ]
    trace = bool(int(os.environ.get("KERNEL_TRACE", "0")))
    kw = {}
    if trace:
        try:
            import trace_support
            trace_support.install()
            tdir = os.environ.get("KERNEL_TRACE_DIR")
            if tdir:
                os.makedirs(tdir, exist_ok=True)
                kw["tmpdir"] = tdir
        except Exception as e:  # tracing is dev-only; never block the run
            print(f"trace support unavailable: {e}")
            trace = False
    r = run_bass_kernel_spmd(nc, in_maps, list(range(N_CORES)), trace=trace, **kw)
    if trace and r.exec_time_ns is not None:
        print(f"HW exec time: {r.exec_time_ns} ns")
    return _combine(r.results)
